# revision 20
# baseline (speedup 1.0000x reference)
"""Categorical cross-entropy loss kernel for Trainium2 (8 NeuronCores).

Computes: out = [-sum(input * log(target + 1e-8)) / B] for input/target of
shape [B=262144, C=128] float32.

Strategy (data-parallel, memory-bound streaming reduction):
  - Shard both tensors along batch across 8 cores (32768 rows each).
  - Each core views its [32768, 128] shard as [128 partitions, 32768 free]
    (partition p owns 256 contiguous rows -> contiguous 128 KiB per
    partition), streams it in 8 chunks of [128, 4096] (2 MiB DMAs).
  - Per chunk: ACT computes log(target + eps) in place, then one fused DVE
    TensorTensorReduce computes input * log_t and its per-partition sum.
  - Per-core output: [128, 8] partial sums; host sums in float64, scales
    by -1/B.
"""

import numpy as np

import concourse.bass as bass
import concourse.tile as tile
from concourse import bacc, mybir
from concourse.bass_utils import run_bass_kernel_spmd

B, C = 262144, 128
NCORES = 8
ROWS = B // NCORES          # 32768 rows per core
P = 128                     # SBUF partitions
FREE = ROWS * C // P        # 32768 f32 per partition
EPS = 1e-8

_NC_CACHE = None


# body chunks stream at full DMA width; the tapered tail shrinks the
# serial ACT->DVE chain after the last byte lands
CH_SCHEDULE = [4096] * 7 + [2048, 1024, 512, 512]
assert sum(CH_SCHEDULE) == FREE


def build_nc(repeat: int = 1, ch_schedule=None, io_bufs: int = 3,
             scratch_bufs: int = 2, inplace_mult: bool = False,
             alt_dma: bool = False, split_rings: bool = False) -> bass.Bass:
    if ch_schedule is None:
        ch_schedule = CH_SCHEDULE
    assert sum(ch_schedule) == FREE
    nch = len(ch_schedule)
    offs = [0]
    for c in ch_schedule:
        offs.append(offs[-1] + c)
    max_ch = max(ch_schedule)
    nc = bacc.Bacc("TRN2", target_bir_lowering=False, debug=False,
                   num_devices=NCORES)
    inp = nc.dram_tensor("input", [ROWS, C], mybir.dt.float32,
                         kind="ExternalInput").ap()
    tgt = nc.dram_tensor("target", [ROWS, C], mybir.dt.float32,
                         kind="ExternalInput").ap()
    out = nc.dram_tensor("out", [P, nch], mybir.dt.float32,
                         kind="ExternalOutput").ap()

    inp_v = inp.rearrange("(p n) c -> p (n c)", p=P)
    tgt_v = tgt.rearrange("(p n) c -> p (n c)", p=P)

    with tile.TileContext(nc) as tc:
        with (
            tc.tile_pool(name="eps", bufs=1) as eps_pool,
            tc.tile_pool(name="io", bufs=io_bufs) as io_pool,
            tc.tile_pool(name="scratch", bufs=scratch_bufs) as scratch_pool,
            tc.tile_pool(name="acc", bufs=1) as acc_pool,
        ):
            # EPS bias for the ACT Ln; Tile tracks the memset->ACT dep so
            # it overlaps the first DMAs (no extra all-engine barrier)
            eps_t = eps_pool.tile([P, 1], mybir.dt.float32)
            nc.gpsimd.memset(eps_t[:], EPS)

            acc = acc_pool.tile([P, nch], mybir.dt.float32)
            for it in range(nch * repeat):
                j = it % nch
                ch = ch_schedule[j]
                dma = nc.scalar if (alt_dma and it % 2) else nc.sync
                dma_inp = nc.scalar if split_rings else dma
                # target first: ACT only needs tgt, so it can start while
                # input is still in flight
                tt = io_pool.tile([P, max_ch], mybir.dt.float32, tag="tgt")
                dma.dma_start(tt[:, :ch], tgt_v[:, offs[j]:offs[j] + ch])
                ti = io_pool.tile([P, max_ch], mybir.dt.float32, tag="inp")
                dma_inp.dma_start(ti[:, :ch], inp_v[:, offs[j]:offs[j] + ch])
                # tt = log(tt + EPS), in place on the ACT engine
                nc.scalar.activation(tt[:, :ch], tt[:, :ch],
                                     mybir.ActivationFunctionType.Ln,
                                     bias=eps_t[:])
                # acc[:, j] = sum_free(ti * tt)
                # (TensorTensorReduce would fuse these, but it crashes the
                # device on this runtime build -- use 2 DVE ops instead)
                if inplace_mult:
                    prod = ti
                else:
                    prod = scratch_pool.tile([P, max_ch], mybir.dt.float32)
                nc.vector.tensor_tensor(prod[:, :ch], ti[:, :ch], tt[:, :ch],
                                        mybir.AluOpType.mult)
                nc.vector.tensor_reduce(acc[:, j:j + 1], prod[:, :ch],
                                        mybir.AxisListType.X,
                                        mybir.AluOpType.add)
            nc.sync.dma_start(out[:], acc[:])
    nc.compile()
    return nc


def shard_inputs(inp: np.ndarray, tgt: np.ndarray) -> list[dict]:
    return [
        {
            "input": np.ascontiguousarray(inp[i * ROWS:(i + 1) * ROWS]),
            "target": np.ascontiguousarray(tgt[i * ROWS:(i + 1) * ROWS]),
        }
        for i in range(NCORES)
    ]


def combine(results: list[dict]) -> np.ndarray:
    total = 0.0
    for r in results:
        total += float(np.sum(np.asarray(r["out"], dtype=np.float64)))
    return np.array([-total / B], dtype=np.float32)


def kernel(**inputs: np.ndarray) -> np.ndarray:
    global _NC_CACHE
    inp = np.ascontiguousarray(np.asarray(inputs["input"], dtype=np.float32))
    tgt = np.ascontiguousarray(np.asarray(inputs["target"], dtype=np.float32))
    assert inp.shape == (B, C) and tgt.shape == (B, C)

    if _NC_CACHE is None:
        _NC_CACHE = build_nc()
    nc = _NC_CACHE

    res = run_bass_kernel_spmd(nc, shard_inputs(inp, tgt),
                               list(range(NCORES)))
    return combine(res.results)


# revision 26
# speedup vs baseline: 1.0596x; 1.0596x over previous
"""Categorical cross-entropy loss kernel for Trainium2 (8 NeuronCores).

Computes: out = [-sum(input * log(target + 1e-8)) / B] for input/target of
shape [B=262144, C=128] float32.

Strategy (data-parallel, memory-bound streaming reduction):
  - Shard both tensors along batch across 8 cores (32768 rows each).
  - Each core views its [32768, 128] shard as [128 partitions, 32768 free]
    (partition p owns 256 contiguous rows -> contiguous 128 KiB per
    partition), streams it in 8 chunks of [128, 4096] (2 MiB DMAs).
  - Per chunk: ACT computes log(target + eps) in place, then one fused DVE
    TensorTensorReduce computes input * log_t and its per-partition sum.
  - Per-core output: [128, 8] partial sums; host sums in float64, scales
    by -1/B.
"""

import numpy as np

import concourse.bass as bass
import concourse.tile as tile
from concourse import bacc, mybir
from concourse.bass_utils import run_bass_kernel_spmd

B, C = 262144, 128
NCORES = 8
ROWS = B // NCORES          # 32768 rows per core
P = 128                     # SBUF partitions
FREE = ROWS * C // P        # 32768 f32 per partition
EPS = 1e-8

_NC_CACHE = None


# body chunks stream at full DMA width; the tapered tail shrinks the
# serial ACT->DVE chain after the last byte lands
CH_SCHEDULE = [4096] * 7 + [2048, 1024, 512, 512]
assert sum(CH_SCHEDULE) == FREE


def build_nc(repeat: int = 1, ch_schedule=None, io_bufs: int = 3,
             scratch_bufs: int = 2, inplace_mult: bool = False,
             alt_dma: bool = False, split_rings: bool = False,
             compute: str = "full") -> bass.Bass:
    if ch_schedule is None:
        ch_schedule = CH_SCHEDULE
    assert sum(ch_schedule) == FREE
    nch = len(ch_schedule)
    offs = [0]
    for c in ch_schedule:
        offs.append(offs[-1] + c)
    max_ch = max(ch_schedule)
    nc = bacc.Bacc("TRN2", target_bir_lowering=False, debug=False,
                   num_devices=NCORES)
    inp = nc.dram_tensor("input", [ROWS, C], mybir.dt.float32,
                         kind="ExternalInput").ap()
    tgt = nc.dram_tensor("target", [ROWS, C], mybir.dt.float32,
                         kind="ExternalInput").ap()
    out = nc.dram_tensor("out", [P, nch], mybir.dt.float32,
                         kind="ExternalOutput").ap()

    inp_v = inp.rearrange("(p n) c -> p (n c)", p=P)
    tgt_v = tgt.rearrange("(p n) c -> p (n c)", p=P)

    with tile.TileContext(nc) as tc:
        with (
            tc.tile_pool(name="eps", bufs=1) as eps_pool,
            tc.tile_pool(name="io", bufs=io_bufs) as io_pool,
            tc.tile_pool(name="scratch", bufs=scratch_bufs) as scratch_pool,
            tc.tile_pool(name="acc", bufs=1) as acc_pool,
        ):
            # EPS bias for the ACT Ln; Tile tracks the memset->ACT dep so
            # it overlaps the first DMAs (no extra all-engine barrier)
            if compute != "none":
                eps_t = eps_pool.tile([P, 1], mybir.dt.float32)
                nc.gpsimd.memset(eps_t[:], EPS)

            acc = None
            if compute == "full":
                acc = acc_pool.tile([P, nch], mybir.dt.float32)
            last_tt = None
            for it in range(nch * repeat):
                j = it % nch
                ch = ch_schedule[j]
                dma = nc.scalar if (alt_dma and it % 2) else nc.sync
                if split_rings == "gpsimd":
                    dma_inp = nc.gpsimd
                elif split_rings:
                    dma_inp = nc.scalar
                else:
                    dma_inp = dma
                # target first: ACT only needs tgt, so it can start while
                # input is still in flight
                tt = io_pool.tile([P, max_ch], mybir.dt.float32, tag="tgt")
                dma.dma_start(tt[:, :ch], tgt_v[:, offs[j]:offs[j] + ch])
                ti = io_pool.tile([P, max_ch], mybir.dt.float32, tag="inp")
                dma_inp.dma_start(ti[:, :ch], inp_v[:, offs[j]:offs[j] + ch])
                last_tt = tt
                if compute == "none":
                    continue
                # tt = log(tt + EPS), in place on the ACT engine
                nc.scalar.activation(tt[:, :ch], tt[:, :ch],
                                     mybir.ActivationFunctionType.Ln,
                                     bias=eps_t[:])
                if compute == "act":
                    continue
                # acc[:, j] = sum_free(ti * tt)
                # (TensorTensorReduce would fuse these, but it crashes the
                # device on this runtime build -- use 2 DVE ops instead)
                if inplace_mult:
                    prod = ti
                else:
                    prod = scratch_pool.tile([P, max_ch], mybir.dt.float32)
                nc.vector.tensor_tensor(prod[:, :ch], ti[:, :ch], tt[:, :ch],
                                        mybir.AluOpType.mult)
                nc.vector.tensor_reduce(acc[:, j:j + 1], prod[:, :ch],
                                        mybir.AxisListType.X,
                                        mybir.AluOpType.add)
            if compute == "full":
                nc.sync.dma_start(out[:], acc[:])
            else:  # timing probes: output is garbage, deps only on last tile
                nc.sync.dma_start(out[:], last_tt[:, :nch])
    nc.compile()
    return nc


def shard_inputs(inp: np.ndarray, tgt: np.ndarray) -> list[dict]:
    return [
        {
            "input": np.ascontiguousarray(inp[i * ROWS:(i + 1) * ROWS]),
            "target": np.ascontiguousarray(tgt[i * ROWS:(i + 1) * ROWS]),
        }
        for i in range(NCORES)
    ]


def combine(results: list[dict]) -> np.ndarray:
    total = 0.0
    for r in results:
        total += float(np.sum(np.asarray(r["out"], dtype=np.float64)))
    return np.array([-total / B], dtype=np.float32)


def kernel(**inputs: np.ndarray) -> np.ndarray:
    global _NC_CACHE
    inp = np.ascontiguousarray(np.asarray(inputs["input"], dtype=np.float32))
    tgt = np.ascontiguousarray(np.asarray(inputs["target"], dtype=np.float32))
    assert inp.shape == (B, C) and tgt.shape == (B, C)

    if _NC_CACHE is None:
        _NC_CACHE = build_nc()
    nc = _NC_CACHE

    res = run_bass_kernel_spmd(nc, shard_inputs(inp, tgt),
                               list(range(NCORES)))
    return combine(res.results)
